# revision 27
# baseline (speedup 1.0000x reference)
"""Trainium2 SPMD kernel for nn_CombinedGeneModel.

Math (per batch b, tech t, gene g; R = T*G independent tiny MLPs):
    h   = relu(x * w1[r,e] + b1[r,e])          e = 0..3
    s   = relu(sum_e h*w2[r,e] + b2[r])
    out = relu(sum_t s[b,t,g]*wg[g,t] + bg[g])

With b1 == 0 (guaranteed by setup_inputs) the E=4 hinge sum folds exactly:
    sum_e w2_e*relu(w1_e*x) = c*relu(x) + d*x
      c = sum_e w2_e*|w1_e|,  d = sum_e w2_e*min(w1_e, 0)
so per row:  s = relu(c*relu(x) + d*x + b2).

Layout: genes on SBUF partitions, batch on the free axis; genes sharded
across the 8 NeuronCores; host pre-transposes x to [G, T, B] fp16 so all
DMA is contiguous.

Engine split per 128-gene tile (free dim = 1024 batch per tech):
  DVE    : q_t = c_t*relu(x_t)        (fused max+mult tensor_scalar)
           r_t = d_t*x_t + q_t        (scalar_tensor_tensor)
           s1  = relu(r1 [+ b21])     -> sb high half (fp16)
  ScalarE: s0  = relu(r0 [+ b20])     -> sb low half  (fp16)
  TensorE: v   = diag(wg0)@s0 + diag(wg1)@s1 -> PSUM fp32 (4 matmuls)
  ScalarE: out = relu(v [+ bg])       PSUM -> fp16 output staging
  GpSimd : builds the diag(wg) stationaries on-device (identity times a
           per-partition scalar; no diag DMA at all) and issues chunked
           SWDGE output stores every 2 tiles.

The walrus build here accepts at most ONE sync wait per instruction; a
post-pass strips transitively-satisfied waits (engines execute their
stream in order, so a wait subsumed by an earlier wait on the same engine
can be dropped) and NoOp-splits any instruction still carrying >1 wait.
"""

import os
import numpy as np

N_GENES = 20000
N_TECH = 2
BATCH = 1024
N_CORES = 8
P = 128
G_PAD = 20480            # next multiple of 8*128 above 20000
GS = G_PAD // N_CORES    # 2560 genes per core
NTILES = GS // P         # 20 tiles of 128 genes
FD = BATCH               # free dim per (tile, tech)
HF = FD // 2             # matmul moving-free-dim limit is 512
NW = 9                   # per-gene scalars: c0,d0,c1,d1,wg0,wg1,b20,b21,bg
STORE_CHUNK = 4          # output store granularity (tiles); 5 stores total
                         # keeps every SWDGE queue (8) virgin -> 1 wait each
N_CHUNKS = (NTILES + STORE_CHUNK - 1) // STORE_CHUNK

LAST_EXEC_NS = None
LAST_RESULTS = None

_nc_cache = {}


def _build_nc(has_b2: bool, has_bg: bool):
    import concourse.bass as bass
    import concourse.mybir as mybir
    from concourse.tile import TileContext

    Op = mybir.AluOpType
    Act = mybir.ActivationFunctionType
    f16 = mybir.dt.float16
    f32 = mybir.dt.float32

    nc = bass.Bass()
    x_d = nc.declare_dram_parameter("x", [NTILES, P, 2 * FD], f16, isOutput=False)
    w_d = nc.declare_dram_parameter("w", [P, NTILES * NW], f32, isOutput=False)
    i_d = nc.declare_dram_parameter("ident", [P, P], f16, isOutput=False)
    o_d = nc.declare_dram_parameter("out", [NTILES, P, FD], f16, isOutput=True)

    with TileContext(nc) as tc:
        with (
            tc.tile_pool(name="wp", bufs=1) as wpool,
            tc.tile_pool(name="xp", bufs=NTILES) as xpool,
            tc.tile_pool(name="qp", bufs=3) as qpool,
            tc.tile_pool(name="r0p", bufs=NTILES) as r0pool,
            tc.tile_pool(name="rp", bufs=3) as rpool,
            tc.tile_pool(name="sp", bufs=4) as spool,
            tc.tile_pool(name="op", bufs=N_CHUNKS) as opool,
            tc.tile_pool(name="ps", bufs=2, space="PSUM") as pspool,
        ):
            w = wpool.tile([P, NTILES * NW], f32)
            nc.sync.dma_start(w[:], w_d[:])
            ident = wpool.tile([P, P], f16)
            nc.sync.dma_start(ident[:], i_d[:])
            # all 40 diag(wg) stationaries live here, built by gpsimd
            dgbuf = wpool.tile([P, NTILES * 2 * P], f16)

            # absorb the w-table DMA wait once per consuming engine so the
            # first real op on each engine carries a single wait
            wt_v = wpool.tile([P, 1], f32)
            nc.vector.tensor_copy(wt_v[:], w[:, 0:1])
            wt_p = wpool.tile([P, 1], f32)
            nc.gpsimd.tensor_tensor(wt_p[:], w[:, 0:1], w[:, 0:1], Op.mult)

            xts = []
            sbs = []
            ochunk = None
            for j in range(NTILES):
                col = j * NW
                c0 = w[:, col + 0 : col + 1]
                d0 = w[:, col + 1 : col + 2]
                c1 = w[:, col + 2 : col + 3]
                d1 = w[:, col + 3 : col + 4]
                wg0 = w[:, col + 4 : col + 5]
                wg1 = w[:, col + 5 : col + 6]
                b20 = w[:, col + 6 : col + 7]
                b21 = w[:, col + 7 : col + 8]
                bg = w[:, col + 8 : col + 9]

                xt = xpool.tile([P, 2 * FD], f16, tag="x")
                nc.sync.dma_start(xt[:], x_d[j])
                xts.append(xt)
                x0 = xt[:, 0:FD]
                x1 = xt[:, FD : 2 * FD]

                # GpSimd: build this tile's two diag stationaries
                # (Pool's standard ucode library has tensor_tensor but not
                # tensor_scalar, so broadcast the [P,1] scalar to [P,P])
                dg0 = dgbuf[:, (2 * j) * P : (2 * j + 1) * P]
                dg1 = dgbuf[:, (2 * j + 1) * P : (2 * j + 2) * P]
                nc.gpsimd.tensor_tensor(dg0, ident[:], wg0.broadcast_to((P, P)),
                                        Op.mult)
                nc.gpsimd.tensor_tensor(dg1, ident[:], wg1.broadcast_to((P, P)),
                                        Op.mult)

                # DVE: q_t = c_t*relu(x_t); r_t = d_t*x_t + q_t
                q0 = qpool.tile([P, FD], f16, tag="q0")
                nc.vector.tensor_scalar(q0[:], x0, 0.0, c0, Op.max, Op.mult)
                r0 = r0pool.tile([P, FD], f16, tag="r0")
                nc.vector.scalar_tensor_tensor(r0[:], x0, d0, q0[:], Op.mult, Op.add)
                q1 = qpool.tile([P, FD], f16, tag="q1")
                nc.vector.tensor_scalar(q1[:], x1, 0.0, c1, Op.max, Op.mult)
                r1 = rpool.tile([P, FD], f16, tag="r1")
                nc.vector.scalar_tensor_tensor(r1[:], x1, d1, q1[:], Op.mult, Op.add)

                # scribble on the outgoing s-tile first: the tiny DVE write
                # carries the PE WAR wait (matmul readers of sb[j-4]) so the
                # real writes into the recycled slot each keep a single wait
                if j >= 4:
                    nc.vector.tensor_copy(sbs[j - 4][:, FD : FD + 1], wt_v[:])

                # s0 on ScalarE (relu, optional +b2), s1 on DVE
                sb = spool.tile([P, 2 * FD], f16, tag="s")
                sbs.append(sb)
                if has_b2:
                    nc.scalar.activation(sb[:, 0:FD], r0[:], Act.Relu, bias=b20)
                    nc.vector.tensor_scalar(sb[:, FD : 2 * FD], r1[:], b21, 0.0,
                                            Op.add, Op.max)
                else:
                    nc.scalar.activation(sb[:, 0:FD], r0[:], Act.Relu)
                    nc.vector.tensor_scalar(sb[:, FD : 2 * FD], r1[:], 0.0, None,
                                            Op.max)

                # TensorE: v = diag(wg0)@s0 + diag(wg1)@s1 (per 512-col half)
                v = pspool.tile([P, FD], f32, tag="v")
                for h in range(2):
                    sl = slice(h * HF, (h + 1) * HF)
                    sl1 = slice(FD + h * HF, FD + (h + 1) * HF)
                    nc.tensor.matmul(v[:, sl], dg0, sb[:, sl],
                                     start=True, stop=False)
                    nc.tensor.matmul(v[:, sl], dg1, sb[:, sl1],
                                     start=False, stop=True)

                # ScalarE: out = relu(v [+ bg]) -> staging chunk
                if j % STORE_CHUNK == 0:
                    ochunk = opool.tile([P, STORE_CHUNK * FD], f16, tag="o")
                oc = ochunk[:, (j % STORE_CHUNK) * FD : (j % STORE_CHUNK + 1) * FD]
                if has_bg:
                    nc.scalar.activation(oc, v[:], Act.Relu, bias=bg)
                else:
                    nc.scalar.activation(oc, v[:], Act.Relu)

                if (j + 1) % STORE_CHUNK == 0 or j == NTILES - 1:
                    k0 = j - j % STORE_CHUNK
                    nt = j + 1 - k0
                    src = ochunk[:, 0 : nt * FD].rearrange(
                        "p (t b) -> p t b", t=nt
                    )
                    dst = o_d[k0 : j + 1].rearrange("t p b -> p t b")
                    nc.gpsimd.dma_start(dst, src)

    _split_multi_waits(nc, mybir)
    return nc


def _split_multi_waits(nc, mybir):
    """walrus (gen3 codegen here) accepts at most one sync wait per
    instruction.  Two rewrites keep every instruction at <=1 wait:

    1. Drop waits that are provably satisfied when the instruction issues:
       engines run their stream in order, so (a) a wait on a semaphore for
       a value some EARLIER instruction on the same engine already waited
       for (>= target) is a no-op, and (b) a wait on the engine's OWN
       semaphore for a value already reached by its own in-stream
       increments is a no-op.  (Tile emits these because its clock
       tracking is not transitive.)
    2. For the remaining multi-wait instructions (e.g. the epilogue Drain,
       which is block-initial), hoist all but one wait onto same-engine
       NoOps appended to the preceding basic block."""
    # KERNEL_SYNC_FIX: 0 = leave Tile's sync untouched, 1 = dedupe +
    # subsume + split the epilogue Drain, 2 = additionally strip every
    # instruction down to a single wait (walrus single-wait mode)
    level = int(os.environ.get("KERNEL_SYNC_FIX", "1"))
    if level <= 0:
        return
    STRIP_TO_ONE = level >= 2
    blocks = list(nc.main_func.blocks)

    # sem id -> set of engines that increment it; sem id -> total increments.
    # Sems with any non-increment update (barrier resets via sem-sub) are
    # non-monotonic: no wait on them may ever be stripped or used as a
    # subsumption witness.
    updaters = {}
    cum_total = {}
    nonmono = set()
    for bb in blocks:
        for ins in bb.instructions:
            si = getattr(ins, "sync_info", None)
            if si is None:
                continue
            for up in si.on_update or []:
                updaters.setdefault(up.id, set()).add(ins.engine)
                if up.update_mode in ("sem-inc", "sem-add-imm"):
                    cum_total[up.id] = cum_total.get(up.id, 0) + up.update_value
                else:
                    nonmono.add(up.id)

    # pass 1: strip satisfied waits, walking in block order while
    # accumulating (per engine) the values already waited-for, and (per
    # sem) the self-increments
    cum = {}          # sem id -> total increments by its own engine stream
    waited = {}       # (engine, sem id) -> max value this engine waited for
    for bb in blocks:
        for ins in bb.instructions:
            si = getattr(ins, "sync_info", None)
            if si is None:
                continue
            waits = list(si.on_wait or [])
            if waits:
                # dedupe waits on the same semaphore (keep the max target)
                best = {}
                merged = []
                for wv in waits:
                    if (
                        wv.sync_type == "semaphore"
                        and wv.wait_mode == "sem-ge-imm"
                        and wv.id not in nonmono
                    ):
                        prev = best.get(wv.id)
                        if prev is None:
                            best[wv.id] = wv
                            merged.append(wv)
                        elif wv.wait_value > prev.wait_value:
                            merged[merged.index(prev)] = wv
                            best[wv.id] = wv
                    else:
                        merged.append(wv)
                waits = merged
                kept = []
                for wv in waits:
                    if (
                        wv.sync_type == "semaphore"
                        and wv.wait_mode == "sem-ge-imm"
                        and wv.id not in nonmono
                    ):
                        # (a) an earlier wait on this engine subsumes it:
                        # that wait observed the semaphore value directly,
                        # so ordering is guaranteed regardless of async
                        # write-landing
                        if waited.get((ins.engine, wv.id), -1) >= wv.wait_value:
                            continue
                        kept.append(wv)
                        waited[(ins.engine, wv.id)] = wv.wait_value
                    else:
                        kept.append(wv)
                # For a still-multi-wait instruction (walrus accepts at
                # most ONE sync wait), keep the single wait that is
                # satisfied LAST: the one with the largest fraction of its
                # semaphore's total increments.  Data dependencies (current
                # tile) outrank WAR/WAW bookkeeping waits (several tiles
                # old), which the tile-pool depth bounds in practice.  The
                # epilogue Drain is exempt; pass 2 splits it onto NoOps.
                if (
                    len(kept) > 1
                    and STRIP_TO_ONE
                    and type(ins).__name__ != "InstDrain"
                    and not any(
                        wv.id in nonmono or wv.sync_type != "semaphore"
                        for wv in kept
                    )
                ):
                    total = {wv.id: max(1, cum_total.get(wv.id, 1))
                             for wv in kept}
                    kept = [max(kept,
                                key=lambda wv: wv.wait_value / total[wv.id])]
                if len(kept) != len(waits):
                    ins.sync_info = mybir.SyncInfo(
                        on_wait=kept, on_update=list(si.on_update or [])
                    )
            si = ins.sync_info
            for up in si.on_update or []:
                if up.update_mode in ("sem-inc", "sem-add-imm"):
                    cum[up.id] = cum.get(up.id, 0) + up.update_value

    # pass 2: NoOp-split anything still multi-wait (the Drain)
    nop_idx = 0
    for bi, bb in enumerate(blocks):
        for ins in bb.instructions:
            si = getattr(ins, "sync_info", None)
            if si is None:
                continue
            waits = list(si.on_wait or [])
            if len(waits) <= 1:
                continue
            assert bi > 0, f"multi-wait instruction in first block: {ins.name}"
            splittable = True
            for other in bb.instructions:
                if other.name == ins.name:
                    break
                if other.engine == ins.engine:
                    splittable = False
                    break
            if not splittable:
                # mid-block multi-wait: leave as-is (hardware accepts
                # multiple waits; only the block-initial Drain with ~20
                # waits needs splitting)
                continue
            prev_bb = blocks[bi - 1]
            for wv in waits[:-1]:
                nop = mybir.InstNoOp(name=f"ant-waitsplit-{nop_idx}")
                nop_idx += 1
                nop.engine = ins.engine
                nop.sync_info = mybir.SyncInfo(on_wait=[wv], on_update=[])
                prev_bb.add_instruction(nop)
            ins.sync_info = mybir.SyncInfo(
                on_wait=[waits[-1]], on_update=list(si.on_update or [])
            )


def _numpy_fallback(x, w1, b1, w2, b2, wg, bgv):
    B = x.shape[0]
    R = N_GENES * N_TECH
    xr = x.reshape(B, R).T.astype(np.float32)
    h = np.maximum(xr[:, :, None] * w1[:, None, :] + b1[:, None, :], 0.0)
    s = np.maximum(np.einsum("rbe,re->rb", h, w2) + b2[:, None], 0.0)
    s = s.T.reshape(B, N_TECH, N_GENES)
    out = np.maximum(np.einsum("btg,gt->bg", s, wg) + bgv, 0.0)
    return out.astype(np.float32)


def kernel(x, weights1, bias1, weights2, bias2, weights_g, bias_g):
    global LAST_EXEC_NS, LAST_RESULTS
    x = np.asarray(x, dtype=np.float32)
    w1 = np.asarray(weights1, dtype=np.float32)
    b1 = np.asarray(bias1, dtype=np.float32)
    w2 = np.asarray(weights2, dtype=np.float32)
    b2 = np.asarray(bias2, dtype=np.float32)
    wg = np.asarray(weights_g, dtype=np.float32)
    bgv = np.asarray(bias_g, dtype=np.float32)

    if np.any(b1 != 0.0):
        # hinge-folding below needs b1 == 0; exact general fallback
        return _numpy_fallback(x, w1, b1, w2, b2, wg, bgv)

    # fold the E=4 expand/shrink into two per-row coefficients
    c = (w2 * np.abs(w1)).sum(axis=1)           # [R]
    d = (w2 * np.minimum(w1, 0.0)).sum(axis=1)  # [R]
    G = N_GENES

    # per-gene scalar table [G_PAD, NW]: c0,d0,c1,d1,wg0,wg1,b20,b21,bg
    wtab = np.zeros((G_PAD, NW), dtype=np.float32)
    wtab[:G, 0] = c[:G]
    wtab[:G, 1] = d[:G]
    wtab[:G, 2] = c[G:]
    wtab[:G, 3] = d[G:]
    wtab[:G, 4] = wg[:, 0]
    wtab[:G, 5] = wg[:, 1]
    wtab[:G, 6] = b2[:G]
    wtab[:G, 7] = b2[G:]
    wtab[:G, 8] = bgv

    # x -> [G_PAD, T, B] fp16, contiguous per gene row
    xt = np.zeros((G_PAD, N_TECH, BATCH), dtype=np.float16)
    xt[:G] = x.transpose(2, 1, 0)

    ident = np.eye(P, dtype=np.float16)
    in_maps = []
    for i in range(N_CORES):
        g0 = i * GS
        xi = np.ascontiguousarray(xt[g0 : g0 + GS].reshape(NTILES, P, 2 * FD))
        wi = np.ascontiguousarray(
            wtab[g0 : g0 + GS].reshape(NTILES, P, NW).transpose(1, 0, 2)
            .reshape(P, NTILES * NW)
        )
        in_maps.append({"x": xi, "w": wi, "ident": ident})

    has_b2 = bool(np.any(b2 != 0.0))
    has_bg = bool(np.any(bgv != 0.0))
    key = (has_b2, has_bg)
    if key not in _nc_cache:
        _nc_cache[key] = _build_nc(has_b2, has_bg)
    nc = _nc_cache[key]

    from concourse.bass_utils import run_bass_kernel_spmd

    trace = bool(int(os.environ.get("KERNEL_TRACE", "0")))
    res = run_bass_kernel_spmd(nc, in_maps, core_ids=list(range(N_CORES)),
                               trace=trace)
    LAST_EXEC_NS = res.exec_time_ns
    LAST_RESULTS = res

    parts = [res.results[i]["out"].reshape(GS, BATCH) for i in range(N_CORES)]
    full = np.concatenate(parts, axis=0)[:G]          # [G, B] fp16
    return np.ascontiguousarray(full.T).astype(np.float32)
